# revision 32
# baseline (speedup 1.0000x reference)
"""Llama4TextExperts MoE expert-parallel kernel for 8 Trainium2 NeuronCores.

Per-core (1 expert each): out_e = (up * silu(gate)) @ W2_e where
[gate|up] = h_e @ W1_e.  All matmuls run in bf16 on the PE array with fp32
PSUM accumulation; SwiGLU is fused into the PSUM eviction of the first
matmul chain.

v6c schedule: the 3072-matmul stream runs at the warm back-to-back floor
(512 cols / 2.4 GHz + 2.5 ns NX = 215.8 ns per matmul, 663 us total);
everything else is startup/tail engineering.  Startup DMAs issue un-chained
in exact consumption order on the single FIFO DMA queue (sync=False
scheduling deps only — completion-chained waves delay *issue* by the
~2-3 us DMA-sem receipt), the w2 slabs are pinned behind mid-mm1 w1 slab
issues so they cannot be hoisted into the startup window, j=0 runs four
interleaved PSUM chains to halve its fresh-hT demand rate, and warmup
matmuls on zeros cover the ~7 us engine-bootstrap preamble + first-chunk
DMA latency so HAM (the PE clock gate) is warm when real work starts.
Startup DMAs are strictly need-ordered (hT k1 before w1 k2:4; w1[0]'s
upper half split so k8 gates on 256 KB).  Measured: 683.2 us +/- 0.3
(seven runs), ~97% of the PE roofline including the ~9.4 us fixed NEFF
preamble/teardown; the residual ~2.6 us of startup stalls are HBM-bound
(j=0's fresh-data demand sits at the ~368 GB/s per-core cap).  NOTE:
runs that catch the chip in the P0 power state execute at 2.0 GHz
instead of 2.4 GHz (~820 us) — environmental, not schedule-dependent
(confirmed: identical binary, 259 ns vs 216 ns matmul spacing).

Host-side prep (not HW time): per-expert slices are cast to bf16 and laid
out in the exact SBUF tiling the device DMAs expect:
  - hT:  h_e transposed to [P, KH, T]   (contraction dim H on partitions)
  - w1:  gate/up column blocks interleaved -> [KI, P, KH, 2P] so each
         128-row gate block sits next to its up block (fused SwiGLU)
  - w2:  [NH, P, KI, 512] slabs (contraction dim I on partitions)
The device output is the natural [T, H] fp32 layout; the host just
concatenates the 8 per-expert results.
"""

import numpy as np

NUM_EXPERTS = 8
HIDDEN = 2048
EXPERT_DIM = 4096
TOK = 1024  # tokens per expert

P = 128
KH = HIDDEN // P        # 16 contraction chunks for matmul 1
KI = EXPERT_DIM // P    # 32 contraction chunks for matmul 2
NT = TOK // 512         # 2  token chunks (psum free dim 512)
NH = HIDDEN // 512      # 4  output-column chunks

TRACE = False           # set by test harness to collect an NTFF profile
LAST_RESULT = None      # BassKernelResults of the most recent run
VARIANT = "v6c"         # kernel schedule variant (see _build_nc)

_NC = {}


def _build_nc(variant):
    import concourse.mybir as mybir
    from concourse import bacc, tile
    from concourse.tile_rust import add_dep_helper

    nc = bacc.Bacc("TRN2", target_bir_lowering=False)
    hT_d = nc.dram_tensor("hT", [P, KH, TOK], mybir.dt.bfloat16, kind="ExternalInput")
    if variant == "v7":
        w1h_d = nc.dram_tensor("w1h", [P, 2, KH, 2 * P], mybir.dt.bfloat16,
                               kind="ExternalInput")
    else:
        w1h_d = None
    w1_d = nc.dram_tensor("w1", [KI, P, KH, 2 * P], mybir.dt.bfloat16, kind="ExternalInput")
    w2_d = nc.dram_tensor("w2", [NH, P, KI, 512], mybir.dt.bfloat16, kind="ExternalInput")
    out_d = nc.dram_tensor("out", [TOK, HIDDEN], mybir.dt.float32, kind="ExternalOutput")

    FT = mybir.dt.float32
    BF = mybir.dt.bfloat16
    KG = 4  # k-chunks per startup DMA (1 MiB hT pieces, 256 KiB w1 pieces)

    if variant.startswith("v6") or variant == "v7":
        _build_v6_body(nc, variant, mybir, tile, add_dep_helper,
                       hT_d, w1_d, w2_d, out_d, w1h_d)
        nc.compile()
        return nc

    with tile.TileContext(nc) as tc:
        with tc.tile_pool(name="ht", bufs=1) as ht_pool, \
             tc.tile_pool(name="w1", bufs=3) as w1_pool, \
             tc.tile_pool(name="gated", bufs=1) as gated_pool, \
             tc.tile_pool(name="w2", bufs=2) as w2_pool, \
             tc.tile_pool(name="tmp", bufs=4) as tmp_pool, \
             tc.tile_pool(name="ob", bufs=4) as ob_pool, \
             tc.tile_pool(name="psum1", bufs=3, space="PSUM") as psum1_pool, \
             tc.tile_pool(name="psum2", bufs=2, space="PSUM") as psum2_pool:

            if variant in ("v4", "v5"):
                # PE warmup: the HAM clock gate runs the PE at 1.2 GHz until
                # it has been busy ~3.4us.  Chew on a zeroed tile while the
                # first input DMAs stream so real matmuls start at 2.4 GHz.
                wz = tmp_pool.tile([P, 512], BF, tag="warm_rhs")
                wl = tmp_pool.tile([P, P], BF, tag="warm_lhs")
                nc.any.memset(wz[:], 0.0)
                nc.any.memset(wl[:], 0.0)
                warm_ps = [psum2_pool.tile([P, 512], FT, tag="po", name=f"warm_{i}")
                           for i in range(2)]
                for i in range(24):
                    nc.tensor.matmul(warm_ps[i % 2][:], wl[:], wz[:],
                                     start=True, stop=True)

            hT = ht_pool.tile([P, KH, TOK], BF)
            gated = gated_pool.tile([P, KI, TOK], BF)

            n_special = 2 if variant == "v4" else 0
            w1t01 = [w1_pool.tile([P, KH, 2 * P], BF, tag="w1t", name=f"w1t_{j}")
                     for j in range(n_special)]

            if variant == "v4":
                # Startup DMAs as coarse chained "waves" in consumption
                # order: concurrent DMAs complete all-together (SDMA
                # round-robins at packet granularity), so unordered the
                # first matmul waits for the LAST startup byte.  Coarse
                # links only — each link costs ~1-2us completion latency.
                w_a = [nc.sync.dma_start(w1t01[0][:], w1_d[0]),
                       nc.sync.dma_start(hT[:, :, 0:512], hT_d[:, :, 0:512])]
                w_b = nc.sync.dma_start(hT[:, :, 512:1024], hT_d[:, :, 512:1024])
                for p in w_a:
                    add_dep_helper(w_b.ins, p.ins, sync=True, reason="wave b")
                w_c = nc.sync.dma_start(w1t01[1][:], w1_d[1])
                add_dep_helper(w_c.ins, w_b.ins, sync=True, reason="wave c")
                prev_wave = [w_c]
            else:
                if n_special:
                    for kg in range(KH // KG):
                        ksl = slice(kg * KG, (kg + 1) * KG)
                        nc.sync.dma_start(w1t01[0][:, ksl, :], w1_d[0, :, ksl, :])
                        nc.sync.dma_start(hT[:, ksl, :], hT_d[:, ksl, :])
                    nc.sync.dma_start(w1t01[1][:], w1_d[1])
                else:
                    for kg in range(KH // KG):
                        ksl = slice(kg * KG, (kg + 1) * KG)
                        nc.sync.dma_start(hT[:, ksl, :], hT_d[:, ksl, :])
                prev_wave = []

            # ---- matmul 1 + fused SwiGLU: gated^T[I, T] ----
            for j in range(KI):
                if j < n_special:
                    # startup: n-outer, gate/up interleaved per k so each
                    # wave's arrival unlocks the next slice of matmuls
                    w1t = w1t01[j]
                    for n in range(NT):
                        tsl = slice(n * 512, (n + 1) * 512)
                        pg = psum1_pool.tile([P, 512], FT, tag="pg", name=f"pg_i{j}_{n}")
                        pu = psum1_pool.tile([P, 512], FT, tag="pu", name=f"pu_i{j}_{n}")
                        for k in range(KH):
                            nc.tensor.matmul(pg[:], w1t[:, k, 0:P], hT[:, k, tsl],
                                             start=(k == 0), stop=(k == KH - 1))
                            nc.tensor.matmul(pu[:], w1t[:, k, P:2 * P], hT[:, k, tsl],
                                             start=(k == 0), stop=(k == KH - 1))
                        sl = tmp_pool.tile([P, 512], BF, tag="silu", name=f"sl_i{j}_{n}")
                        nc.scalar.activation(sl[:], pg[:], mybir.ActivationFunctionType.Silu)
                        nc.vector.tensor_mul(out=gated[:, j, tsl], in0=sl[:], in1=pu[:])
                    continue
                w1t = w1_pool.tile([P, KH, 2 * P], BF, tag="w1t")
                di = nc.sync.dma_start(w1t[:], w1_d[j])
                if j == n_special and prev_wave:
                    # keep this slab load out of the startup waves' bandwidth
                    for p in prev_wave:
                        add_dep_helper(di.ins, p.ins, sync=True, reason="after startup waves")
                for n in range(NT):
                    tsl = slice(n * 512, (n + 1) * 512)
                    pg = psum1_pool.tile([P, 512], FT, tag="pg")
                    pu = psum1_pool.tile([P, 512], FT, tag="pu")
                    for k in range(KH):
                        nc.tensor.matmul(pg[:], w1t[:, k, 0:P], hT[:, k, tsl],
                                         start=(k == 0), stop=(k == KH - 1))
                    for k in range(KH):
                        nc.tensor.matmul(pu[:], w1t[:, k, P:2 * P], hT[:, k, tsl],
                                         start=(k == 0), stop=(k == KH - 1))
                    sl = tmp_pool.tile([P, 512], BF, tag="silu")
                    nc.scalar.activation(sl[:], pg[:], mybir.ActivationFunctionType.Silu)
                    nc.vector.tensor_mul(out=gated[:, j, tsl], in0=sl[:], in1=pu[:])

            # ---- matmul 2: out[T, H] = gated @ W2 ----
            for hc in range(NH):
                w2t = w2_pool.tile([P, KI, 512], BF)
                nc.sync.dma_start(w2t[:], w2_d[hc])
                for t in range(TOK // P):
                    po = psum2_pool.tile([P, 512], FT, tag="po")
                    for i in range(KI):
                        nc.tensor.matmul(po[:], gated[:, i, t * P:(t + 1) * P],
                                         w2t[:, i, :],
                                         start=(i == 0), stop=(i == KI - 1))
                    ob = ob_pool.tile([P, 512], FT, tag="ob")
                    nc.vector.tensor_copy(ob[:], po[:])
                    nc.sync.dma_start(out_d[t * P:(t + 1) * P, hc * 512:(hc + 1) * 512], ob[:])

    nc.compile()
    return nc


def _build_v6_body(nc, variant, mybir, tile, add_dep_helper,
                   hT_d, w1_d, w2_d, out_d, w1h_d=None):
    """v6 schedule.

    Startup fixes over v4 (from NTFF trace analysis):
      - v4's sync=True DMA wave chain serialized *issue* on *completion*
        (hT's 2nd half issued at t=19us, w1[1] at t=35us), while the
        dep-free w2 slabs got hoisted to t=8.6/19.7us and ate the
        startup HBM bandwidth.  All DMAs land on one FIFO queue, so
        plain issue order == transfer order: v6 orders startup DMAs in
        exact consumption order with sync=False (scheduling-only) deps
        and pins the w2 slabs behind mid-mm1 w1 slab issues.
      - j=0 runs 4 interleaved PSUM chains (gate/up x both token
        halves per k-step) so its fresh-hT consumption rate (~2x
        slower per k-chunk) matches single-queue HBM delivery.
      - w1[0]/w1[1] are fetched as half-slabs interleaved between hT
        chunks for just-in-time arrival.
      - Warmup matmul count sized to cover the ~7us runtime preamble +
        first-chunk DMA latency, keeping HAM warm without delaying the
        first real matmul.
    """
    FT = mybir.dt.float32
    BF = mybir.dt.bfloat16
    NWARM = {"v6": 16, "v6b": 12, "v6e": 12}.get(variant, 14)

    # v7 packs the j0/j1 w1 slab pair into SBUF (+16KB/partition), paid
    # for by shallower w1/tmp/ob rings; v6* keeps the deeper rings.
    w1b, tmpb, obb = (2, 3, 3) if variant == "v7" else (3, 4, 4)
    with tile.TileContext(nc) as tc:
        with tc.tile_pool(name="ht", bufs=1) as ht_pool, \
             tc.tile_pool(name="w1", bufs=w1b) as w1_pool, \
             tc.tile_pool(name="gated", bufs=1) as gated_pool, \
             tc.tile_pool(name="w2", bufs=2) as w2_pool, \
             tc.tile_pool(name="tmp", bufs=tmpb) as tmp_pool, \
             tc.tile_pool(name="ob", bufs=obb) as ob_pool, \
             tc.tile_pool(name="psum1", bufs=3, space="PSUM") as psum1_pool, \
             tc.tile_pool(name="psum2", bufs=2, space="PSUM") as psum2_pool:

            # PE warmup: HAM clock-gates the PE to 1.2 GHz until it has
            # been busy ~3.4us; chew on zeros while the startup DMAs run.
            wz = tmp_pool.tile([P, 512], BF, tag="warm_rhs")
            wl = tmp_pool.tile([P, P], BF, tag="warm_lhs")
            nc.vector.memset(wz[:], 0.0)
            nc.vector.memset(wl[:], 0.0)
            warm_ps = [psum2_pool.tile([P, 512], FT, tag="po", name=f"warm_{i}")
                       for i in range(2)]
            for i in range(NWARM):
                nc.tensor.matmul(warm_ps[i % 2][:], wl[:], wz[:],
                                 start=True, stop=True)

            hT = ht_pool.tile([P, KH, TOK], BF)
            gated = gated_pool.tile([P, KI, TOK], BF)
            if variant == "v7":
                w1t01 = []
            else:
                w1t01 = [w1_pool.tile([P, KH, 2 * P], BF, tag="w1t",
                                      name=f"w1t_{j}")
                         for j in range(2)]

            # Startup DMAs in exact consumption order on one FIFO queue.
            # Ultra-fine head: the first real matmul's gate is a minimal
            # transfer so the ~3us DMA-sem receipt delay costs little and
            # real compute starts ~12.5us.  (Splitting hT onto the second
            # HWDGE ring was tried and lost ~5us — keep one ring.)
            H8 = KH // 2
            if variant == "v7":
                # Paired-j0/j1 startup: groups consume w1h (j-interleaved
                # slab pair) + hT token-half n=0 first, so fresh-data
                # demand is ~296 GB/s — under the ~368 GB/s HBM cap —
                # and every chunk sem beats its consumption deadline.
                w1p = ht_pool.tile([P, 2, KH, 2 * P], BF, tag="w1p")
                startup = []
                for a, b in [(0, 1), (1, 2), (2, 3), (3, 4), (4, 6),
                             (6, 8), (8, 10), (10, 12), (12, 14), (14, 16)]:
                    startup.append(nc.sync.dma_start(
                        w1p[:, :, a:b, :], w1h_d[:, :, a:b, :]))
                    startup.append(nc.sync.dma_start(
                        hT[:, a:b, 0:512], hT_d[:, a:b, 0:512]))
                startup.append(nc.sync.dma_start(hT[:, 0:8, 512:1024],
                                                 hT_d[:, 0:8, 512:1024]))
                startup.append(nc.sync.dma_start(hT[:, 8:16, 512:1024],
                                                 hT_d[:, 8:16, 512:1024]))
                for a2, b2 in zip(startup, startup[1:]):
                    add_dep_helper(b2.ins, a2.ins, sync=False,
                                   reason="startup order")
                last_dma = startup[-1]
            else:
                w1p = None
                # strict need-order: hT k1 before w1 k2:4 (k1's gate), and
                # w1[0]'s upper half split so k8 waits only 256 KB.
                startup = [
                    nc.sync.dma_start(w1t01[0][:, 0:2, :], w1_d[0, :, 0:2, :]),
                    nc.sync.dma_start(hT[:, 0:1, :], hT_d[:, 0:1, :]),
                    nc.sync.dma_start(hT[:, 1:2, :], hT_d[:, 1:2, :]),
                    nc.sync.dma_start(w1t01[0][:, 2:4, :], w1_d[0, :, 2:4, :]),
                    nc.sync.dma_start(hT[:, 2:4, :], hT_d[:, 2:4, :]),
                    nc.sync.dma_start(w1t01[0][:, 4:H8, :], w1_d[0, :, 4:H8, :]),
                    nc.sync.dma_start(hT[:, 4:6, :], hT_d[:, 4:6, :]),
                    nc.sync.dma_start(hT[:, 6:8, :], hT_d[:, 6:8, :]),
                    nc.sync.dma_start(w1t01[0][:, H8:12, :], w1_d[0, :, H8:12, :]),
                    nc.sync.dma_start(hT[:, 8:10, :], hT_d[:, 8:10, :]),
                    nc.sync.dma_start(hT[:, 10:12, :], hT_d[:, 10:12, :]),
                    nc.sync.dma_start(w1t01[0][:, 12:KH, :], w1_d[0, :, 12:KH, :]),
                    nc.sync.dma_start(w1t01[1][:, 0:H8, :], w1_d[1, :, 0:H8, :]),
                    nc.sync.dma_start(hT[:, 12:14, :], hT_d[:, 12:14, :]),
                    nc.sync.dma_start(hT[:, 14:16, :], hT_d[:, 14:16, :]),
                    nc.sync.dma_start(w1t01[1][:, H8:KH, :], w1_d[1, :, H8:KH, :]),
                ]
                for a2, b2 in zip(startup, startup[1:]):
                    add_dep_helper(b2.ins, a2.ins, sync=False,
                                   reason="startup order")
                last_dma = startup[-1]

            # ---- matmul 1 + fused SwiGLU: gated^T[I, T] ----
            w1_dmas = {}
            for j in range(KI):
                if variant == "v7" and j < 2:
                    if j == 1:
                        continue
                    # j=0 and j=1 as one pair, token-half n groups: four
                    # interleaved chains per group, stationary weights from
                    # the j-interleaved w1p pair.
                    for n in range(NT):
                        tsl = slice(n * 512, (n + 1) * 512)
                        pgs = [psum1_pool.tile([P, 512], FT, tag="pg",
                                               name=f"pg7_{n}_{jj}")
                               for jj in range(2)]
                        pus = [psum1_pool.tile([P, 512], FT, tag="pu",
                                               name=f"pu7_{n}_{jj}")
                               for jj in range(2)]
                        for k in range(KH):
                            for jj in range(2):
                                nc.tensor.matmul(pgs[jj][:],
                                                 w1p[:, jj, k, 0:P],
                                                 hT[:, k, tsl],
                                                 start=(k == 0),
                                                 stop=(k == KH - 1))
                                nc.tensor.matmul(pus[jj][:],
                                                 w1p[:, jj, k, P:2 * P],
                                                 hT[:, k, tsl],
                                                 start=(k == 0),
                                                 stop=(k == KH - 1))
                        for jj in range(2):
                            sl = tmp_pool.tile([P, 512], BF, tag="silu",
                                               name=f"sl7_{n}_{jj}")
                            nc.scalar.activation(
                                sl[:], pgs[jj][:],
                                mybir.ActivationFunctionType.Silu)
                            nc.vector.tensor_mul(out=gated[:, jj, tsl],
                                                 in0=sl[:], in1=pus[jj][:])
                    continue
                if j == 0:
                    # 4 interleaved chains: per k-step, gate/up for both
                    # token halves (n-pairs reuse the stationary weights
                    # and halve the fresh-hT consumption rate).
                    w1t = w1t01[0]
                    pg = [psum1_pool.tile([P, 512], FT, tag="pg", name=f"pg0_{n}")
                          for n in range(NT)]
                    pu = [psum1_pool.tile([P, 512], FT, tag="pu", name=f"pu0_{n}")
                          for n in range(NT)]
                    for k in range(KH):
                        for n in range(NT):
                            tsl = slice(n * 512, (n + 1) * 512)
                            nc.tensor.matmul(pg[n][:], w1t[:, k, 0:P],
                                             hT[:, k, tsl],
                                             start=(k == 0), stop=(k == KH - 1))
                        for n in range(NT):
                            tsl = slice(n * 512, (n + 1) * 512)
                            nc.tensor.matmul(pu[n][:], w1t[:, k, P:2 * P],
                                             hT[:, k, tsl],
                                             start=(k == 0), stop=(k == KH - 1))
                    for n in range(NT):
                        tsl = slice(n * 512, (n + 1) * 512)
                        sl = tmp_pool.tile([P, 512], BF, tag="silu",
                                           name=f"sl0_{n}")
                        nc.scalar.activation(sl[:], pg[n][:],
                                             mybir.ActivationFunctionType.Silu)
                        nc.vector.tensor_mul(out=gated[:, 0, tsl], in0=sl[:],
                                             in1=pu[n][:])
                    continue
                if j == 1:
                    w1t = w1t01[1]
                else:
                    w1t = w1_pool.tile([P, KH, 2 * P], BF, tag="w1t")
                    di = nc.sync.dma_start(w1t[:], w1_d[j])
                    add_dep_helper(di.ins, last_dma.ins, sync=False,
                                   reason="w1 slab order")
                    last_dma = di
                    w1_dmas[j] = di
                for n in range(NT):
                    tsl = slice(n * 512, (n + 1) * 512)
                    pg = psum1_pool.tile([P, 512], FT, tag="pg")
                    pu = psum1_pool.tile([P, 512], FT, tag="pu")
                    for k in range(KH):
                        nc.tensor.matmul(pg[:], w1t[:, k, 0:P], hT[:, k, tsl],
                                         start=(k == 0), stop=(k == KH - 1))
                    for k in range(KH):
                        nc.tensor.matmul(pu[:], w1t[:, k, P:2 * P], hT[:, k, tsl],
                                         start=(k == 0), stop=(k == KH - 1))
                    sl = tmp_pool.tile([P, 512], BF, tag="silu")
                    nc.scalar.activation(sl[:], pg[:],
                                         mybir.ActivationFunctionType.Silu)
                    nc.vector.tensor_mul(out=gated[:, j, tsl], in0=sl[:],
                                         in1=pu[:])

            # ---- matmul 2: out[T, H] = gated @ W2 ----
            for hc in range(NH):
                w2t = w2_pool.tile([P, KI, 512], BF)
                dw = nc.sync.dma_start(w2t[:], w2_d[hc])
                if hc < 2:
                    # keep the 4 MiB w2 slabs out of the startup window
                    anchor = w1_dmas[20 if hc == 0 else 24]
                    add_dep_helper(dw.ins, anchor.ins, sync=False,
                                   reason="w2 after mid-mm1 w1 slab")
                for t in range(TOK // P):
                    tsl = slice(t * P, (t + 1) * P)
                    if hc == NH - 1 and t == TOK // P - 1:
                        # Tail: two N=256 half-chains so the first half's
                        # eviction + store DMA hides under the second
                        # half's matmuls, shortening the post-last-matmul
                        # drain.
                        for half in range(2):
                            csl = slice(half * 256, (half + 1) * 256)
                            po = psum2_pool.tile([P, 256], FT, tag="po",
                                                 name=f"tail_{half}")
                            for i in range(KI):
                                nc.tensor.matmul(po[:], gated[:, i, tsl],
                                                 w2t[:, i, csl],
                                                 start=(i == 0),
                                                 stop=(i == KI - 1))
                            ob = ob_pool.tile([P, 256], FT, tag="ob",
                                              name=f"tob_{half}")
                            nc.vector.tensor_copy(ob[:], po[:])
                            nc.sync.dma_start(
                                out_d[tsl, hc * 512 + half * 256:
                                      hc * 512 + (half + 1) * 256], ob[:])
                        continue
                    po = psum2_pool.tile([P, 512], FT, tag="po")
                    for i in range(KI):
                        nc.tensor.matmul(po[:], gated[:, i, tsl],
                                         w2t[:, i, :],
                                         start=(i == 0), stop=(i == KI - 1))
                    ob = ob_pool.tile([P, 512], FT, tag="ob")
                    nc.vector.tensor_copy(ob[:], po[:])
                    nc.sync.dma_start(out_d[tsl,
                                            hc * 512:(hc + 1) * 512], ob[:])


def _get_nc():
    if VARIANT not in _NC:
        _NC[VARIANT] = _build_nc(VARIANT)
    return _NC[VARIANT]


def kernel(hidden_states, gate_up_proj, down_proj):
    import ml_dtypes
    from concourse.bass_utils import run_bass_kernel_spmd

    global LAST_RESULT
    bf16 = ml_dtypes.bfloat16

    h = np.asarray(hidden_states, dtype=np.float32)
    w1 = np.asarray(gate_up_proj, dtype=np.float32)
    w2 = np.asarray(down_proj, dtype=np.float32)
    assert h.shape == (NUM_EXPERTS * TOK, HIDDEN)
    assert w1.shape == (NUM_EXPERTS, HIDDEN, 2 * EXPERT_DIM)
    assert w2.shape == (NUM_EXPERTS, EXPERT_DIM, HIDDEN)

    nc = _get_nc()

    in_maps = []
    for e in range(NUM_EXPERTS):
        he = h[e * TOK:(e + 1) * TOK]                       # [T, H]
        # [H, T] -> [KH, P, T] -> [P, KH, T]
        hT_e = he.T.reshape(KH, P, TOK).transpose(1, 0, 2).astype(bf16)
        # [H, 2I]: col = gu*I + j*P + m -> [j, p, ko, gu*P + m]
        w1_e = (w1[e].reshape(KH, P, 2, KI, P)
                .transpose(3, 1, 0, 2, 4)
                .reshape(KI, P, KH, 2 * P)
                .astype(bf16))
        # [I, H]: row = ki*P + p, col = hc*512 + c -> [hc, p, ki, c]
        w2_e = (w2[e].reshape(KI, P, NH, 512)
                .transpose(2, 1, 0, 3)
                .reshape(NH, P, KI, 512)
                .astype(bf16))
        im = {"hT": hT_e, "w1": w1_e, "w2": w2_e}
        if VARIANT == "v7":
            # j0/j1 slab pair, j-interleaved: [P, 2, KH, 2P]
            im["w1h"] = np.ascontiguousarray(w1_e[0:2].transpose(1, 0, 2, 3))
        in_maps.append(im)

    res = run_bass_kernel_spmd(nc, in_maps, list(range(NUM_EXPERTS)), trace=TRACE)
    LAST_RESULT = res

    out = np.concatenate([res.results[e]["out"] for e in range(NUM_EXPERTS)], axis=0)
    return out.astype(np.float32)

